# revision 1
# baseline (speedup 1.0000x reference)
"""Trainium2 Bass kernel for 4-head spatial attention score softmax.

Reference computation:
    qk = einsum('bcxy,oc->boxy', fmap[1,256,64,64], W_qk[1024,256])
    q, k = split(qk, 2, axis=1)             # each [1, 512, 64, 64]
    q = q reshaped to heads, scaled by 128^-0.5
    sim[b,h,xy,uv] = q . k  (contraction over dim_head=128)
    out = softmax(sim, axis=-1)             # [1, 4, 4096, 4096] f32

Sharding: 8 cores = 4 heads x 2 query-halves. Each core projects q for its
2048 query columns + k for all 4096 columns, computes scores with fp16
matmuls, softmax, and streams a [2048, 4096] bf16 slab to HBM (host upcasts
to f32; bf16 rounding is ~0.1% rms vs the 2e-2 gate).

Per-core inputs are fp16 with the core's OWN query half as columns [0:2048]
(odd cores get the two 2048-column halves swapped) so the q projection uses
a static offset 0 and runs while the later fmap chunks still load. The host
un-swaps the output columns of odd cores.

softmax engine split per 128-query tile (4096 columns, 2 PSUM halves):
  - ScalarE: exp on cols [FQ:2048] of each half (bf16 out + accumulated
    row partial sums).
  - DVE: cols [0:FQ] of each half via a bit-trick fast exp2: in f32,
    t = s*(128/ln2) + (127*128 - C + 1.5*2^23); the magic addend forces
    round-to-integer in the mantissa, so the low 16 bits of t are exactly
    the bf16 encoding of a linear-interp exp (~1.8% rms error on these
    columns, ~0.8% overall; inputs are deterministic). A second DVE op
    compacts the strided low-16-bit lanes into the bf16 output row and
    accumulates those partial sums for free.
  - DVE: 4->1 partial-sum reduce, reciprocal, one 4x-mode 4096-wide bf16
    normalize multiply; DMA streams the row block out.

Hardware notes (from perfetto traces of ~15 variants):
  - The PE clock needs ~3us of dense matmul activity to reach 2.4 GHz and
    falls back to 1.2 GHz after idle gaps; warmup matmuls bridge the input
    load so the projections run at speed.
  - Back-to-back DMAs on one HWDGE queue interleave descriptors
    round-robin across the 16 DMA engines, so both input chunks would
    complete together; a tiny gating read serializes chunk 1 behind
    chunk 0 for the early projections.
  - tensor_scalar with accum_out runs at 1x on HW regardless of dtype
    (the 4x mode applies only without accumulation), which is why the
    fast-exp keeps the separate magic+compact pair, and why the fast
    columns cost ~2.08 ns/col on DVE vs 0.98 on ScalarE: FQ=384 balances
    the two engines at ~3.6-3.7us busy per tile.
  - A dummy activation at program start pulls the 1.3us Exp table load
    into ScalarE's idle input-load window.
  - Steady state is coupled at ~4.4us/tile by PSUM double-buffering (2 x
    [128,2048] f32 fills all 8 banks): the next tile's matmuls only start
    when the previous exp releases its half, and measured exec time is
    ~16us of fixed program overhead + the modeled ~90us of work.
"""

import numpy as np

import concourse.bacc as bacc
import concourse.mybir as mybir
import concourse.tile as tile
from concourse import bass_utils

HEADS = 4
DIM_HEAD = 128
C = 256          # input channels
XY = 4096        # 64*64 spatial positions
QCHUNK = 2048    # query positions per core
N_CORES = 8
SCALE = DIM_HEAD ** -0.5

F32 = mybir.dt.float32
BF16 = mybir.dt.bfloat16
F16 = mybir.dt.float16

# fast-exp constants: low 16 bits of (s*A + B) in f32 are the bf16 encoding
# of ~exp(s) (linear-interp exp2, ~1.8% rms, tuned mean-unbiased by C_FE).
# The 1.5*2^23 magic addend forces round-to-integer inside the f32 mantissa.
# NOTE: a direct int16-converting tensor_scalar would skip the compaction
# pass, but on HW any tensor_scalar with accum_out runs at 1x, so the
# separate magic+compact pair is the cheapest row-sum-carrying form.
FE_A = float(np.float32(128.0 / np.log(2.0)))
C_FE = 7.0
FE_B = float(np.float32(127.0 * 128.0 - C_FE + 12582912.0))  # + 1.5*2^23

FQ = 384         # fast-exp columns per 2048-wide half


def _emit(tc, fmap_k, wqkt, out):
    nc = tc.nc

    with tc.tile_pool(name="consts", bufs=1) as consts:
        # Weights transposed on host: [c, d] with c split into 2 partition
        # chunks. wqkt = [wq.T | wk.T] concatenated: one DMA instead of two.
        w_sb = consts.tile([128, 2, 2 * DIM_HEAD], F16)
        # fmap [256, n] -> [128p, 2, n]
        fk_sb = consts.tile([128, 2, XY], F16)
        warm_sb = consts.tile([128, 512], F16)
        junk = consts.tile([128, 4], F16)
        fk_src = fmap_k.rearrange("(a p) n -> p a n", p=128)
        nc.sync.dma_start(out=w_sb, in_=wqkt.rearrange("(a p) d -> p a d", p=128))
        # fmap in two 2048-column chunks; each chunk's partition-groups go
        # to the two HWDGE queues (SP + Activation) concurrently. Chunk 1
        # is gated behind a tiny read spanning both chunks' SBUF cells:
        # descriptors of back-to-back DMAs on a queue interleave
        # round-robin across the DMA engines, so without the gate chunk 0's
        # completion only fires when ALL input bytes are done (~4us later),
        # stalling the early projections.
        nc.sync.dma_start(out=fk_sb[:, 0, 0:2048], in_=fk_src[:, 0, 0:2048])
        nc.scalar.dma_start(out=fk_sb[:, 1, 0:2048], in_=fk_src[:, 1, 0:2048])
        nc.vector.memset(fk_sb[:, 0:2, 2048:2049], 0.0)  # init the gate cells
        nc.vector.tensor_copy(junk, fk_sb[:, 0:2, 2047:2049])
        nc.sync.dma_start(out=fk_sb[:, 0, 2048:XY], in_=fk_src[:, 0, 2048:XY])
        nc.scalar.dma_start(out=fk_sb[:, 1, 2048:XY], in_=fk_src[:, 1, 2048:XY])

        q_sb = consts.tile([128, QCHUNK], F16)  # [d, x] for this core's queries
        k_sb = consts.tile([128, XY], F16)      # [d, uv]

        nc.vector.memset(warm_sb, 0.0)
        # dummy activation right away: the 1.3us Exp ACT_TABLE_LOAD fires
        # here, in ScalarE's idle input-load window, not inside the first
        # real exp on the critical path.
        tbl = consts.tile([128, 1], BF16)
        nc.scalar.activation(out=tbl, in_=warm_sb[:, 0:1],
                             func=mybir.ActivationFunctionType.Exp)

        # One PSUM pool + tag for warmup, projections, and scores: a second
        # pool would overlap the first's banks and pick up a release
        # dependency on the *last* projection, stalling the first score
        # matmuls behind work they don't need.
        with tc.tile_pool(name="ps", bufs=2, space="PSUM") as ps_pool, \
             tc.tile_pool(name="soft", bufs=12) as soft_pool, \
             tc.tile_pool(name="ft", bufs=8) as ft_pool, \
             tc.tile_pool(name="small", bufs=10) as small_pool:
            # PE warmup: dummy matmuls with no load deps keep TensorE busy
            # through the input-DMA window, ramping the HAM clock (a PE
            # idle gap drops it back to 1.2 GHz).
            warm_ps = ps_pool.tile([128, 2048], F32, tag="ps")
            for i in range(12):
                nc.tensor.matmul(warm_ps[:, 0:512], lhsT=warm_sb[:, 0:128],
                                 rhs=warm_sb, start=True, stop=True)

            # ---- k projection for one 2048-col chunk g. PSUM->SBUF copies
            # interleaved per 512 across ScalarE+DVE so the PSUM buffer
            # frees quickly and each copy lands right after its matmuls.
            def emit_kproj(g):
                ps_k = ps_pool.tile([128, 2048], F32, tag="ps",
                                    name=f"ps_k{g}")
                for j in range(4):
                    osl = slice(j * 512, (j + 1) * 512)
                    ksl = slice(g * 2048 + j * 512, g * 2048 + (j + 1) * 512)
                    nc.tensor.matmul(ps_k[:, osl],
                                     lhsT=w_sb[:, 0, DIM_HEAD:2 * DIM_HEAD],
                                     rhs=fk_sb[:, 0, ksl],
                                     start=True, stop=False)
                    nc.tensor.matmul(ps_k[:, osl],
                                     lhsT=w_sb[:, 1, DIM_HEAD:2 * DIM_HEAD],
                                     rhs=fk_sb[:, 1, ksl],
                                     start=False, stop=True)
                    if j % 2 == 0:
                        nc.scalar.copy(k_sb[:, ksl], ps_k[:, osl])
                    else:
                        nc.vector.tensor_copy(k_sb[:, ksl], ps_k[:, osl])

            # ---- q projection, 1024 columns per call (cols are fmap cols
            # [0:2048) by host reorder = chunk 0: static offset, no wait on
            # the second fmap chunk).
            def emit_qproj(cq):
                ps_q = ps_pool.tile([128, 2048], F32, tag="ps",
                                    name=f"ps_q{cq}")
                for j in range(2):
                    osl = slice(cq * 1024 + j * 512, cq * 1024 + (j + 1) * 512)
                    nc.tensor.matmul(
                        ps_q[:, osl], lhsT=w_sb[:, 0, 0:DIM_HEAD],
                        rhs=fk_sb[:, 0, osl],
                        start=True, stop=False)
                    nc.tensor.matmul(
                        ps_q[:, osl], lhsT=w_sb[:, 1, 0:DIM_HEAD],
                        rhs=fk_sb[:, 1, osl],
                        start=False, stop=True)
                nc.scalar.copy(q_sb[:, cq * 1024:cq * 1024 + 512],
                               ps_q[:, cq * 1024:cq * 1024 + 512])
                nc.vector.tensor_copy(q_sb[:, cq * 1024 + 512:(cq + 1) * 1024],
                                      ps_q[:, cq * 1024 + 512:(cq + 1) * 1024])

            # ---- per-tile pieces ----
            ets = {}
            pps = {}
            dens = {}

            def emit_half(qt, half):
                if qt not in ets:
                    ets[qt] = soft_pool.tile([128, XY], BF16, tag="et",
                                             name=f"et{qt}")
                    pps[qt] = small_pool.tile([128, 4], F32, tag="pp",
                                              name=f"pp{qt}")
                    dens[qt] = small_pool.tile([128, 1], F32, tag="den",
                                               name=f"den{qt}")
                et, pp = ets[qt], pps[qt]
                qsl = q_sb[:, qt * 128:(qt + 1) * 128]
                ps = ps_pool.tile([128, 2048], F32, tag="ps",
                                  name=f"ps_t{qt}h{half}")
                # The fast-exp columns sit at the END of the half, in their
                # own matmul: ScalarE's exp then reads exactly the first
                # four matmuls' range [0:2048-FQ], so the next tile's
                # j0..j3 never overlap the fast region and the exp's wait
                # chain is one matmul shorter.
                E = 2048 - FQ
                for a, b in ((0, 512), (512, 1024), (1024, 1536),
                             (1536, E), (E, 2048)):
                    if a == b:
                        continue
                    nc.tensor.matmul(ps[:, a:b], lhsT=qsl,
                                     rhs=k_sb[:, half * 2048 + a:
                                              half * 2048 + b],
                                     start=True, stop=True)
                # DVE fast-exp on cols [E:2048], then compact the strided
                # low-16-bit lanes into the bf16 output row with the row
                # partial sum for free.
                ft = ft_pool.tile([128, FQ], F32, tag="ft")
                nc.vector.tensor_scalar(
                    out=ft, in0=ps[:, E:2048],
                    scalar1=FE_A, scalar2=FE_B,
                    op0=mybir.AluOpType.mult, op1=mybir.AluOpType.add)
                nc.vector.tensor_scalar(
                    out=et[:, half * 2048 + E:(half + 1) * 2048],
                    in0=ft.bitcast(BF16)[:, 0:2 * FQ:2],
                    scalar1=1.0, scalar2=None,
                    op0=mybir.AluOpType.mult, op1=mybir.AluOpType.add,
                    accum_out=pp[:, 2 * half + 1:2 * half + 2])
                # ScalarE exp on cols [0:E]
                nc.scalar.activation(
                    out=et[:, half * 2048:half * 2048 + E],
                    in_=ps[:, 0:E],
                    func=mybir.ActivationFunctionType.Exp,
                    accum_out=pp[:, 2 * half:2 * half + 1])

            def emit_norm_store(qt, nsplit=1):
                et, pp, den = ets[qt], pps[qt], dens[qt]
                nc.vector.tensor_reduce(den, pp[:, 0:4],
                                        axis=mybir.AxisListType.X,
                                        op=mybir.AluOpType.add)
                nc.vector.reciprocal(den, den)
                w = XY // nsplit
                for i in range(nsplit):
                    sl = slice(i * w, (i + 1) * w)
                    nc.vector.tensor_scalar_mul(et[:, sl], et[:, sl], den)
                    nc.sync.dma_start(out=out[qt * 128:(qt + 1) * 128, sl],
                                      in_=et[:, sl])

            # ---- schedule. Projections run as the fmap chunks land; the
            # first tile's h0 half starts once k[0:2048] is projected,
            # before fmap chunk 1 even arrives.
            # PSUM allocation order (bufs=2 alternate A/B): warm(A) qp0(B)
            # kp0(A) h00(B) qp1(A) kp1(B) h01(A) h10(B) h11(A) h20(B)...
            # Every allocation's buffer predecessor releases before (or
            # right when) its own data dependencies are ready, so the
            # 2-deep PSUM rotation never adds serialization. qproj(1) fills
            # the PE gap while fmap chunk 1 is still in flight.
            emit_qproj(0)        # q cols 0:1024 (tiles 0-7), fmap chunk 0
            emit_kproj(0)        # k cols 0:2048, fmap chunk 0
            emit_half(0, 0)      # scores vs k[0:2048]
            emit_qproj(1)        # q cols 1024:2048 (tiles 8-15), chunk 0
            emit_kproj(1)        # k cols 2048:4096, fmap chunk 1
            emit_half(0, 1)
            emit_half(1, 0)
            emit_norm_store(0)
            emit_half(1, 1)
            # Steady tiles flow with a one-half lookahead: tile t+1's first
            # half is emitted BEFORE tile t's normalize+store so DVE has
            # work queued while waiting for ScalarE's second-half row sums.
            NT = QCHUNK // 128
            for qt in range(2, NT):
                emit_half(qt, 0)
                emit_norm_store(qt - 1)
                emit_half(qt, 1)
            emit_norm_store(NT - 1, nsplit=4)


def build_program():
    nc = bacc.Bacc("TRN2", target_bir_lowering=False, debug=False,
                   enable_asserts=False)
    fmap_k = nc.dram_tensor("fmap_k", [C, XY], F16, kind="ExternalInput").ap()
    wqkt = nc.dram_tensor("wqkt", [C, 2 * DIM_HEAD], F16,
                          kind="ExternalInput").ap()
    out = nc.dram_tensor("out", [QCHUNK, XY], BF16, kind="ExternalOutput").ap()

    with tile.TileContext(nc) as tc:
        _emit(tc, fmap_k, wqkt, out)
    nc.compile()
    return nc


_CACHE = {}


def _get_nc():
    if "nc" not in _CACHE:
        _CACHE["nc"] = build_program()
    return _CACHE["nc"]


def make_in_maps(fmap, W_qk):
    fm = np.asarray(fmap, dtype=np.float32).reshape(C, XY)
    # per-query-half column orders: own half first
    fm_h = [np.ascontiguousarray(fm.astype(np.float16)),
            np.ascontiguousarray(
                np.concatenate([fm[:, QCHUNK:], fm[:, :QCHUNK]],
                               axis=1).astype(np.float16))]
    W = np.asarray(W_qk, dtype=np.float32)
    in_maps = []
    for core in range(N_CORES):
        hd, qhalf = divmod(core, 2)
        wq = W[hd * DIM_HEAD:(hd + 1) * DIM_HEAD] * np.float32(SCALE)
        wk = W[HEADS * DIM_HEAD + hd * DIM_HEAD:
               HEADS * DIM_HEAD + (hd + 1) * DIM_HEAD]
        in_maps.append({
            "fmap_k": fm_h[qhalf],
            "wqkt": np.ascontiguousarray(
                np.concatenate([wq.T, wk.T], axis=1).astype(np.float16)),
        })
    return in_maps


def assemble(per_core_outs):
    out = np.empty((HEADS, XY, XY), dtype=np.float32)
    for core in range(N_CORES):
        hd, qhalf = divmod(core, 2)
        slab = np.asarray(per_core_outs[core]).astype(np.float32)
        if qhalf == 1:
            # core's k columns were [2048:4096 | 0:2048]: un-swap
            slab = np.concatenate([slab[:, QCHUNK:], slab[:, :QCHUNK]], axis=1)
        out[hd, qhalf * QCHUNK:(qhalf + 1) * QCHUNK, :] = slab
    return out.reshape(1, HEADS, XY, XY)


def kernel(fmap, W_qk, trace=False):
    nc = _get_nc()
    in_maps = make_in_maps(fmap, W_qk)
    res = bass_utils.run_bass_kernel_spmd(
        nc, in_maps, core_ids=list(range(N_CORES)), trace=trace)
    out = assemble([res.results[c]["out"] for c in range(N_CORES)])
    if trace:
        kernel.last_exec_time_ns = res.exec_time_ns
        kernel.last_results = res
    return out

